# revision 16
# baseline (speedup 1.0000x reference)
"""Multi-head attention (no mask) Trainium2 kernel, SPMD over 8 NeuronCores.

Problem: x[2,2048,1024] @ wq/wk/wv[1024,1024] (+zero biases) -> 16-head
scaled-dot-product attention (softmax over full sequence, no causal mask),
output [2,2048,1024] fp32.

Sharding: batch x head-group. Core i handles batch i//4 and heads
4*(i%4)..4*(i%4)+4 (256 output columns). Host concatenates per batch.

Per-core pipeline (matmuls bf16, fp32 PSUM):
  1. x (fp32 DRAM) --SWDGE cast--> x16 (bf16 DRAM scratch)
  2. x16 --HWDGE DMA transpose--> xT [128 D-part, 2048 s] tiles
  3. per head pair P (2 pairs of 2 heads): qT/kT/vT [128, 2048] =
     w_slice.T @ xT
  4. vT --PE transpose--> v65[P] [128 kseq, 16 kchunk, 130]:
     [0:64]=v_h0, [64]=1, [65:129]=v_h1, [129]=1
  5. per (qc of 512 q, pair): for kc of 128 kseq (2-group lookahead):
       scoresT[k,q] = kT.T @ qT per head -- the two heads ride PE row
       groups (0-63 / 64-127) as adjacent instructions so they execute
       concurrently in the PE array;
       exp alternates between ScalarE (exact Exp activation) and DVE
       (Schraudolph: att_bits_i16 = round(s*23.083 + 16250.5), bitcast
       bf16 == exp(s/8) within ~3%);
       yT[65,512] += v65.T @ attT  (row 64 accumulates the denominator)
  6. finalize per (qc, pair): PE-transpose yT -> [128 q, 65], DVE
     reciprocal of col 64, ScalarE scale-by-reciprocal copy into yo,
     one DMA per qc.
"""

import os
import sys

import numpy as np

for _p in ("/opt/trn_rl_repo", "/root/.axon_site/_ro/trn_rl_repo"):
    if _p not in sys.path and os.path.isdir(_p):
        sys.path.append(_p)

from contextlib import ExitStack

import concourse.bass as bass
import concourse.tile as tile
from concourse import bacc, library_config, masks, mybir
from concourse.bass_utils import run_bass_kernel_spmd

FP32 = mybir.dt.float32
BF16 = mybir.dt.bfloat16
I16 = mybir.dt.int16

N_CORES = 8
B, S, D = 2, 2048, 1024
COLS = 256            # output columns per core = 4 heads x 64
HD = 64               # head dim
NP = 2                # head pairs per core
SCALE = 0.125         # 1 / sqrt(HD)
QCH = 512             # q chunk (psum free dim)
KCH = 128             # k chunk (partition dim)
NKC = S // KCH        # 16
NQC = S // QCH        # 4
NJ = QCH // 128       # 4 transpose blocks per q chunk
DT = D // 128         # 8 contraction tiles for projections

# Schraudolph fast-exp constants: bf16 bits of exp(s/8) ~ s*A + Bc.
EXP_A = 0.125 * 1.4426950408889634 * 128.0   # 23.0831...
EXP_B = 16250.9  # tuned for the DVE's truncating fp32->int16 conversion
# Which kc groups run exp on DVE (the rest use ScalarE's exact Exp).
DVE_KC = frozenset((1, 3, 5, 7, 9, 11))

_CACHED_NC = {}


def build_nc(reps=1, zero_bias=True):
    nc = bacc.Bacc("TRN2", target_bir_lowering=False, debug=False,
                   num_devices=N_CORES)

    x = nc.dram_tensor("x", [S, D], FP32, kind="ExternalInput").ap()
    w_ap = {}
    b_ap = {}
    for p in ("q", "k", "v"):
        w_ap[p] = nc.dram_tensor(f"w{p}", [D, COLS], FP32,
                                 kind="ExternalInput").ap()
        b_ap[p] = nc.dram_tensor(f"b{p}", [COLS], FP32,
                                 kind="ExternalInput").ap()
    out = nc.dram_tensor("out", [S, COLS], FP32, kind="ExternalOutput").ap()

    with tile.TileContext(nc) as tc, ExitStack() as ctx:
        dram_pool = ctx.enter_context(tc.tile_pool(name="dram", bufs=1,
                                                   space="DRAM"))
        const_pool = ctx.enter_context(tc.tile_pool(name="const", bufs=1))
        w_pool = ctx.enter_context(tc.tile_pool(name="w", bufs=1))
        xt_pool = ctx.enter_context(tc.tile_pool(name="xt", bufs=2))
        qkv_pool = ctx.enter_context(tc.tile_pool(name="qkv", bufs=2))
        v65_pool = ctx.enter_context(tc.tile_pool(name="v65", bufs=2))
        att_pool = ctx.enter_context(tc.tile_pool(name="att", bufs=4))
        fin_pool = ctx.enter_context(tc.tile_pool(name="fin", bufs=4))
        yout_pool = ctx.enter_context(tc.tile_pool(name="yout", bufs=2))
        ps_pool = ctx.enter_context(tc.tile_pool(name="psbig", bufs=3,
                                                 space="PSUM"))
        psy_pool = ctx.enter_context(tc.tile_pool(name="psy", bufs=1,
                                                  space="PSUM"))

        nc.gpsimd.load_library(library_config.attn)

        id_f32 = const_pool.tile([128, 128], FP32, tag="idf")
        id_bf16 = const_pool.tile([128, 128], BF16, tag="idb")
        masks.make_identity(nc, id_f32[:])
        masks.make_identity(nc, id_bf16[:])

        # Weights (cast fp32->bf16 during DMA) and biases [128, pair].
        w_sb = {}
        b_sb = {}
        for p in ("q", "k", "v"):
            wt = w_pool.tile([128, DT, COLS], BF16, tag=f"w{p}")
            for t in range(DT):
                nc.gpsimd.dma_start(out=wt[:, t, :],
                                    in_=w_ap[p][t * 128:(t + 1) * 128, :])
            w_sb[p] = wt
            bt = w_pool.tile([128, NP], FP32, tag=f"b{p}")
            nc.sync.dma_start(out=bt[:],
                              in_=b_ap[p].rearrange("(u p) -> p u", u=NP))
            b_sb[p] = bt

        # bf16 copy of x in DRAM (enables the 2-byte HWDGE DMA transpose).
        x16 = dram_pool.tile([S, D], BF16, name="x16")
        state = {}  # per-rep tiles; cleared each rep

        def emit_cast():
            for c in range(4):
                nc.gpsimd.dma_start(
                    out=x16[c * 512:(c + 1) * 512, :].rearrange(
                        "s (u v) -> (s u) v", u=4),
                    in_=x[c * 512:(c + 1) * 512, :].rearrange(
                        "s (u v) -> (s u) v", u=4))

        def emit_xt(half):
            if "xt" not in state:
                state["xt"] = xt_pool.tile([128, DT, S], BF16, tag="xt",
                                           name="xt")
            xt = state["xt"]
            x16v = x16.rearrange("(hh s) (t p) -> hh s t p", p=128, hh=2)
            for t in range(DT):
                nc.sync.dma_start(out=xt[:, t, half * 1024:(half + 1) * 1024],
                                  in_=x16v[half, :, t], transpose=True)

        def emit_proj(p, pair, sc):
            if (p, pair) not in state:
                state[p, pair] = qkv_pool.tile([128, S], BF16,
                                               tag=f"{p}T{pair}",
                                               name=f"{p}T{pair}")
            pt = state[p, pair]
            xt = state["xt"]
            ps = ps_pool.tile([128, QCH], FP32, tag="big", name="psproj")
            for t in range(DT):
                nc.tensor.matmul(
                    ps[:], lhsT=w_sb[p][:, t, pair * 128:(pair + 1) * 128],
                    rhs=xt[:, t, sc * QCH:(sc + 1) * QCH],
                    start=(t == 0), stop=(t == DT - 1))
            dst = pt[:, sc * QCH:(sc + 1) * QCH]
            if zero_bias:
                if p == "k":
                    nc.scalar.activation(
                        dst, ps[:], mybir.ActivationFunctionType.Copy)
                else:
                    nc.vector.tensor_copy(dst, ps[:])
            else:
                nc.vector.tensor_scalar_add(
                    dst, ps[:], b_sb[p][:, pair:pair + 1])

        def emit_v65(pair, kc0, n):
            if ("v65", pair) not in state:
                v65 = v65_pool.tile([128, NKC, 130], BF16, tag=f"v65{pair}",
                                    name="v65")
                nc.vector.memset(v65[:, :, 64], 1.0)
                nc.vector.memset(v65[:, :, 129], 1.0)
                state["v65", pair] = v65
            v65 = state["v65", pair]
            vt = state["v", pair]
            for kc in range(kc0, kc0 + n):
                pvt = ps_pool.tile([128, 128], BF16, tag="big", name="psvt")
                nc.tensor.transpose(pvt[:], vt[:, kc * 128:(kc + 1) * 128],
                                    id_bf16[:])
                # one copy, dst columns {0:64, 65:129}
                nc.vector.tensor_copy(
                    v65[:, kc, :].rearrange("p (u f) -> p u f", u=2)[:, :, 0:64],
                    pvt[:].rearrange("p (u f) -> p u f", u=2))

        def emit_attn_qc(qc, pair, hook):
            qT, kT = state["q", pair], state["k", pair]
            v65 = state["v65", pair]
            psy = psy_pool.tile([65, NP, QCH], FP32, tag="psy", name="psy")

            pss_q = {}

            def scores(kc):
                pss = ps_pool.tile([128, NP, QCH], FP32, tag="big",
                                   name="pss")
                for h in range(2):
                    nc.tensor.matmul(
                        pss[:, h, :],
                        lhsT=kT[h * HD:(h + 1) * HD,
                                kc * 128:(kc + 1) * 128],
                        rhs=qT[h * HD:(h + 1) * HD,
                               qc * QCH:(qc + 1) * QCH],
                        start=True, stop=True)
                pss_q[kc] = pss

            scores(0)
            scores(1)
            for kc in range(NKC):
                if kc + 2 < NKC:
                    scores(kc + 2)
                pss = pss_q.pop(kc)
                if kc in DVE_KC:
                    at = att_pool.tile([128, NP, QCH], I16, tag="att",
                                       name="att")
                    nc.vector.tensor_scalar(
                        at[:], pss[:], EXP_A, EXP_B,
                        op0=mybir.AluOpType.mult, op1=mybir.AluOpType.add)
                    at_bf = at[:].bitcast(BF16)
                else:
                    at = att_pool.tile([128, NP, QCH], BF16, tag="att",
                                       name="att")
                    nc.scalar.activation(
                        at[:], pss[:],
                        mybir.ActivationFunctionType.Exp, scale=SCALE)
                    at_bf = at[:]
                for h in range(2):
                    nc.tensor.matmul(
                        psy[:, h, :],
                        lhsT=v65[:, kc, h * 65:(h + 1) * 65],
                        rhs=at_bf[:, h, :],
                        start=(kc == 0), stop=(kc == NKC - 1))
                if hook is not None:
                    hook()

            # Finalize: yT -> transpose -> normalize (recip on DVE, scaled
            # copy on ScalarE) -> yo; DMA once per qc after both pairs.
            if ("yo", qc) not in state:
                state["yo", qc] = yout_pool.tile([128, NJ, COLS], FP32,
                                                 tag="yo", name="yo")
            yo = state["yo", qc]
            for h in range(2):
                ysb = fin_pool.tile([65, QCH], FP32, tag="ysb", name="ysb")
                if h == 0:
                    nc.vector.tensor_copy(ysb[:], psy[:, h, :])
                else:
                    nc.scalar.activation(
                        ysb[:], psy[:, h, :],
                        mybir.ActivationFunctionType.Copy)
                for j in range(NJ):
                    pyt = ps_pool.tile([128, 65], FP32, tag="big",
                                       name="psyt")
                    nc.tensor.transpose(pyt[:], ysb[:, j * 128:(j + 1) * 128],
                                        id_f32[0:65, 0:65])
                    yn = fin_pool.tile([128, 65], FP32, tag="yn", name="yn")
                    nc.vector.tensor_copy(yn[:], pyt[:])
                    nc.gpsimd.normalize_recip(
                        yo[:, j, pair * 128 + h * HD:
                           pair * 128 + (h + 1) * HD],
                        yn[:, 0:64], yn[:, 64:65])
            if pair == NP - 1:
                nc.sync.dma_start(
                    out=out[qc * QCH:(qc + 1) * QCH, :].rearrange(
                        "(j p) c -> p j c", p=128),
                    in_=yo[:])

        for _rep in range(reps):
            state.clear()
            emit_cast()
            emit_xt(0)
            emit_xt(1)
            # Pair-0 prologue: k, v (for v65), q slice 0.
            for sc in range(NQC):
                emit_proj("k", 0, sc)
            for sc in range(NQC):
                emit_proj("v", 0, sc)
            emit_proj("q", 0, 0)
            emit_v65(0, 0, NKC)

            # Remaining prologue work, interleaved into attention sweeps.
            pending = []
            for sc in range(NQC):
                pending.append(("k1", lambda sc=sc: emit_proj("k", 1, sc)))
            for sc in range(NQC):
                pending.append(("v1", lambda sc=sc: emit_proj("v", 1, sc)))
            for kc0 in range(0, NKC, 4):
                pending.append(
                    ("v651", lambda kc0=kc0: emit_v65(1, kc0, 4)))
            pending.append(("q1s0", lambda: emit_proj("q", 1, 0)))
            pending.append(("q0s1", lambda: emit_proj("q", 0, 1)))
            pending.append(("q1s1", lambda: emit_proj("q", 1, 1)))
            pending.append(("q0s2", lambda: emit_proj("q", 0, 2)))
            pending.append(("q1s2", lambda: emit_proj("q", 1, 2)))
            pending.append(("q0s3", lambda: emit_proj("q", 0, 3)))
            pending.append(("q1s3", lambda: emit_proj("q", 1, 3)))
            pending.reverse()
            done = set()

            def hook():
                if pending:
                    key, fn = pending.pop()
                    fn()
                    done.add(key)

            def require_upto(key):
                # pending is in dependency order; popping up to `key`
                # drains every prerequisite before it too.
                while pending and key not in done:
                    hook()

            for qc in range(NQC):
                for pair in range(NP):
                    if pair == 1:
                        require_upto(f"q1s{qc}")
                    elif qc > 0:
                        require_upto(f"q0s{qc}")
                    emit_attn_qc(qc, pair, hook)
            while pending:
                hook()

    nc.compile()
    return nc


def get_nc(zero_bias=True):
    if zero_bias not in _CACHED_NC:
        _CACHED_NC[zero_bias] = build_nc(zero_bias=zero_bias)
    return _CACHED_NC[zero_bias]


def make_in_maps(x, wq, bq, wk, bk, wv, bv):
    in_maps = []
    for i in range(N_CORES):
        b = i // 4
        c0 = (i % 4) * COLS
        in_maps.append({
            "x": np.ascontiguousarray(x[b], dtype=np.float32),
            "wq": np.ascontiguousarray(wq[:, c0:c0 + COLS], dtype=np.float32),
            "wk": np.ascontiguousarray(wk[:, c0:c0 + COLS], dtype=np.float32),
            "wv": np.ascontiguousarray(wv[:, c0:c0 + COLS], dtype=np.float32),
            "bq": np.ascontiguousarray(bq[c0:c0 + COLS], dtype=np.float32),
            "bk": np.ascontiguousarray(bk[c0:c0 + COLS], dtype=np.float32),
            "bv": np.ascontiguousarray(bv[c0:c0 + COLS], dtype=np.float32),
        })
    return in_maps


def kernel(x, wq, bq, wk, bk, wv, bv):
    zero_bias = not (np.any(bq) or np.any(bk) or np.any(bv))
    nc = get_nc(zero_bias)
    in_maps = make_in_maps(x, wq, bq, wk, bk, wv, bv)
    res = run_bass_kernel_spmd(nc, in_maps, list(range(N_CORES)))
    out = np.empty((B, S, D), dtype=np.float32)
    for i in range(N_CORES):
        b = i // 4
        c0 = (i % 4) * COLS
        out[b, :, c0:c0 + COLS] = res.results[i]["out"]
    kernel.last_results = res
    return out


# revision 33
# speedup vs baseline: 1.1232x; 1.1232x over previous
"""Multi-head attention (no mask) Trainium2 kernel, SPMD over 8 NeuronCores.

Problem: x[2,2048,1024] @ wq/wk/wv[1024,1024] (+zero biases) -> 16-head
scaled-dot-product attention (softmax over full sequence, no causal mask),
output [2,2048,1024] fp32.

Sharding: batch x head-group. Core i handles batch i//4 and heads
4*(i%4)..4*(i%4)+4 (256 output columns). Host concatenates per batch.

Per-core pipeline (matmuls bf16, fp32 PSUM):
  1. x (fp32 DRAM) --SWDGE cast--> x16 (bf16 DRAM scratch)
  2. x16 --HWDGE DMA transpose--> xT [128 D-part, 2048 s] tiles
  3. per head pair P (2 pairs of 2 heads): qT/kT/vT [128, 2048] =
     w_slice.T @ xT
  4. vT --PE transpose--> v65[P] [128 kseq, 16 kchunk, 130]:
     [0:64]=v_h0, [64]=1, [65:129]=v_h1, [129]=1
  5. per (qc of 512 q, pair): for kc of 128 kseq (2-group lookahead):
       scoresT[k,q] = kT.T @ qT per head -- the two heads ride PE row
       groups (0-63 / 64-127) as adjacent instructions so they execute
       concurrently in the PE array;
       exp alternates between ScalarE (exact Exp activation) and DVE
       (Schraudolph: att_bits_i16 = round(s*23.083 + 16250.5), bitcast
       bf16 == exp(s/8) within ~3%);
       yT[65,512] += v65.T @ attT  (row 64 accumulates the denominator)
  6. finalize per (qc, pair): PE-transpose yT -> [128 q, 65], DVE
     reciprocal of col 64, ScalarE scale-by-reciprocal copy into yo,
     one DMA per qc.
"""

import os
import sys

import numpy as np

for _p in ("/opt/trn_rl_repo", "/root/.axon_site/_ro/trn_rl_repo"):
    if _p not in sys.path and os.path.isdir(_p):
        sys.path.append(_p)

from contextlib import ExitStack

import concourse.bass as bass
import concourse.tile as tile
from concourse import bacc, library_config, masks, mybir
from concourse.bass_utils import run_bass_kernel_spmd

FP32 = mybir.dt.float32
BF16 = mybir.dt.bfloat16
I16 = mybir.dt.int16

N_CORES = 8
B, S, D = 2, 2048, 1024
COLS = 256            # output columns per core = 4 heads x 64
HD = 64               # head dim
NP = 2                # head pairs per core
SCALE = 0.125         # 1 / sqrt(HD)
QCH = 512             # q chunk (psum free dim)
KCH = 128             # k chunk (partition dim)
NKC = S // KCH        # 16
NQC = S // QCH        # 4
NJ = QCH // 128       # 4 transpose blocks per q chunk
DT = D // 128         # 8 contraction tiles for projections

# Schraudolph fast-exp constants: bf16 bits of exp(s/8) ~ s*A + Bc.
EXP_A = 0.125 * 1.4426950408889634 * 128.0   # 23.0831...
EXP_B = 16250.9  # tuned for the DVE's truncating fp32->int16 conversion
# Which kc groups run exp on DVE (the rest use ScalarE's exact Exp).
DVE_KC = frozenset((1, 3, 5, 7, 9, 11, 13))
# k64 arch: which of the 8 kc-pair groups per sweep run exp on DVE.
DVE_G = frozenset((1, 4, 6))

_CACHED_NC = {}


def build_nc(reps=1, zero_bias=True, dve_kc=None, fin_mode="scalar",
             arch="k64"):
    dve_kc = DVE_KC if dve_kc is None else frozenset(dve_kc)
    nc = bacc.Bacc("TRN2", target_bir_lowering=False, debug=False,
                   num_devices=N_CORES)

    x = nc.dram_tensor("x", [S, D], FP32, kind="ExternalInput").ap()
    w_ap = {}
    b_ap = {}
    for p in ("q", "k", "v"):
        w_ap[p] = nc.dram_tensor(f"w{p}", [D, COLS], FP32,
                                 kind="ExternalInput").ap()
        b_ap[p] = nc.dram_tensor(f"b{p}", [COLS], FP32,
                                 kind="ExternalInput").ap()
    out = nc.dram_tensor("out", [S, COLS], FP32, kind="ExternalOutput").ap()

    with tile.TileContext(nc) as tc, ExitStack() as ctx:
        dram_pool = ctx.enter_context(tc.tile_pool(name="dram", bufs=2,
                                                   space="DRAM"))
        const_pool = ctx.enter_context(tc.tile_pool(name="const", bufs=1))
        w_pool = ctx.enter_context(tc.tile_pool(name="w", bufs=1))
        xt_pool = ctx.enter_context(tc.tile_pool(name="xt", bufs=2))
        qkv_pool = ctx.enter_context(tc.tile_pool(name="qkv", bufs=2))
        v65_pool = ctx.enter_context(tc.tile_pool(name="v65", bufs=2))
        att_pool = ctx.enter_context(
            tc.tile_pool(name="att", bufs=(20 if arch == "k64" else 4)))
        fin_pool = ctx.enter_context(tc.tile_pool(name="fin", bufs=4))
        yout_pool = ctx.enter_context(tc.tile_pool(name="yout", bufs=2))
        ps_pool = ctx.enter_context(tc.tile_pool(name="psbig", bufs=3,
                                                 space="PSUM"))
        psy_pool = ctx.enter_context(tc.tile_pool(name="psy", bufs=1,
                                                  space="PSUM"))

        if fin_mode == "pool":
            nc.gpsimd.load_library(library_config.attn)

        id_f32 = const_pool.tile([128, 128], FP32, tag="idf")
        id_bf16 = const_pool.tile([128, 128], BF16, tag="idb")
        masks.make_identity(nc, id_f32[:])
        masks.make_identity(nc, id_bf16[:])

        # Weights (cast fp32->bf16 during DMA) and biases [128, pair].
        w_sb = {}
        b_sb = {}
        for p in ("q", "k", "v"):
            wt = w_pool.tile([128, DT, COLS], BF16, tag=f"w{p}")
            for t in range(DT):
                nc.gpsimd.dma_start(out=wt[:, t, :],
                                    in_=w_ap[p][t * 128:(t + 1) * 128, :])
            w_sb[p] = wt
            bt = w_pool.tile([128, NP], FP32, tag=f"b{p}")
            nc.sync.dma_start(out=bt[:],
                              in_=b_ap[p].rearrange("(u p) -> p u", u=NP))
            b_sb[p] = bt

        state = {}  # per-rep tiles; cleared each rep

        def emit_cast():
            # bf16 copy of x in DRAM (enables 2-byte HWDGE DMA transpose).
            x16 = dram_pool.tile([S, D], BF16, tag="x16", name="x16")
            state["x16"] = x16
            for c in range(4):
                nc.gpsimd.dma_start(
                    out=x16[c * 512:(c + 1) * 512, :].rearrange(
                        "s (u v) -> (s u) v", u=4),
                    in_=x[c * 512:(c + 1) * 512, :].rearrange(
                        "s (u v) -> (s u) v", u=4))

        def emit_xt(half):
            if "xt" not in state:
                state["xt"] = xt_pool.tile([128, DT, S], BF16, tag="xt",
                                           name="xt")
            xt = state["xt"]
            x16v = state["x16"].rearrange("(hh s) (t p) -> hh s t p",
                                          p=128, hh=2)
            for t in range(DT):
                nc.sync.dma_start(out=xt[:, t, half * 1024:(half + 1) * 1024],
                                  in_=x16v[half, :, t], transpose=True)

        def emit_proj(p, pair, sc):
            if (p, pair) not in state:
                state[p, pair] = qkv_pool.tile([128, S], BF16,
                                               tag=f"{p}T{pair}",
                                               name=f"{p}T{pair}")
            pt = state[p, pair]
            xt = state["xt"]
            ps = ps_pool.tile([128, QCH], FP32, tag="big", name="psproj")
            for t in range(DT):
                nc.tensor.matmul(
                    ps[:], lhsT=w_sb[p][:, t, pair * 128:(pair + 1) * 128],
                    rhs=xt[:, t, sc * QCH:(sc + 1) * QCH],
                    start=(t == 0), stop=(t == DT - 1))
            dst = pt[:, sc * QCH:(sc + 1) * QCH]
            if zero_bias:
                if p == "k":
                    nc.scalar.activation(
                        dst, ps[:], mybir.ActivationFunctionType.Copy)
                else:
                    nc.vector.tensor_copy(dst, ps[:])
            else:
                nc.vector.tensor_scalar_add(
                    dst, ps[:], b_sb[p][:, pair:pair + 1])

        def emit_v65(pair, kc0, n):
            if ("v65", pair) not in state:
                v65 = v65_pool.tile([128, NKC, 130], BF16, tag=f"v65{pair}",
                                    name="v65")
                nc.vector.memset(v65[:, :, 64], 1.0)
                nc.vector.memset(v65[:, :, 129], 1.0)
                state["v65", pair] = v65
            v65 = state["v65", pair]
            vt = state["v", pair]
            for kc in range(kc0, kc0 + n):
                pvt = ps_pool.tile([128, 128], BF16, tag="big", name="psvt")
                nc.tensor.transpose(pvt[:], vt[:, kc * 128:(kc + 1) * 128],
                                    id_bf16[:])
                # one copy, dst columns {0:64, 65:129}
                nc.vector.tensor_copy(
                    v65[:, kc, :].rearrange("p (u f) -> p u f", u=2)[:, :, 0:64],
                    pvt[:].rearrange("p (u f) -> p u f", u=2))

        def emit_exp(pss, shape, use_dve):
            """exp(scores) -> att (bf16 view), on ScalarE or DVE."""
            if use_dve:
                at = att_pool.tile(shape, I16, tag="att", name="att")
                nc.vector.tensor_scalar(
                    at[:], pss[:], EXP_A, EXP_B,
                    op0=mybir.AluOpType.mult, op1=mybir.AluOpType.add)
                return at[:].bitcast(BF16)
            at = att_pool.tile(shape, BF16, tag="att", name="att")
            nc.scalar.activation(
                at[:], pss[:], mybir.ActivationFunctionType.Exp, scale=SCALE)
            return at[:]

        def finalize(pair, make_ysb, yo, nj, qch):
            for h in range(2):
                ysb = make_ysb(h, fin_pool.tile([65, qch], FP32, tag="ysb",
                                                name="ysb"))
                for j in range(nj):
                    pyt = ps_pool.tile([128, 65], FP32, tag="big",
                                       name="psyt")
                    nc.tensor.transpose(pyt[:], ysb[:, j * 128:(j + 1) * 128],
                                        id_f32[0:65, 0:65])
                    dst = yo[:, j, pair * 128 + h * HD:
                             pair * 128 + (h + 1) * HD]
                    if fin_mode == "pool":
                        yn = fin_pool.tile([128, 65], FP32, tag="yn",
                                           name="yn")
                        nc.vector.tensor_copy(yn[:], pyt[:])
                        nc.gpsimd.normalize_recip(
                            dst, yn[:, 0:64], yn[:, 64:65])
                    else:
                        rc = fin_pool.tile([128, 1], FP32, tag="rc",
                                           name="rc")
                        nc.vector.reciprocal(rc[:], pyt[:, 64:65])
                        nc.scalar.activation(
                            dst, pyt[:, 0:64],
                            mybir.ActivationFunctionType.Copy, scale=rc[:])

        def emit_attn_qc(qc, pair, hook):
            qT, kT = state["q", pair], state["k", pair]
            v65 = state["v65", pair]
            psy = psy_pool.tile([65, NP, QCH], FP32, tag="psy", name="psy")

            pss_q = {}

            def scores(kc):
                pss = ps_pool.tile([128, NP, QCH], FP32, tag="big",
                                   name="pss")
                for h in range(2):
                    nc.tensor.matmul(
                        pss[:, h, :],
                        lhsT=kT[h * HD:(h + 1) * HD,
                                kc * 128:(kc + 1) * 128],
                        rhs=qT[h * HD:(h + 1) * HD,
                               qc * QCH:(qc + 1) * QCH],
                        start=True, stop=True)
                pss_q[kc] = pss

            scores(0)
            scores(1)
            for kc in range(NKC):
                if kc + 2 < NKC:
                    scores(kc + 2)
                at_bf = emit_exp(pss_q.pop(kc), [128, NP, QCH], kc in dve_kc)
                for h in range(2):
                    nc.tensor.matmul(
                        psy[:, h, :],
                        lhsT=v65[:, kc, h * 65:(h + 1) * 65],
                        rhs=at_bf[:, h, :],
                        start=(kc == 0), stop=(kc == NKC - 1))
                if hook is not None:
                    hook()

            if ("yo", qc) not in state:
                state["yo", qc] = yout_pool.tile([128, NJ, COLS], FP32,
                                                 tag="yo", name="yo")
            yo = state["yo", qc]

            def make_ysb(h, ysb):
                if h == 0:
                    nc.vector.tensor_copy(ysb[:], psy[:, h, :])
                else:
                    nc.scalar.activation(
                        ysb[:], psy[:, h, :],
                        mybir.ActivationFunctionType.Copy)
                return ysb

            finalize(pair, make_ysb, yo, NJ, QCH)
            if pair == NP - 1:
                nc.sync.dma_start(
                    out=out[qc * QCH:(qc + 1) * QCH, :].rearrange(
                        "(j p) c -> p j c", p=128),
                    in_=yo[:])

        # ---- k64 arch: half-phase pipelined sweeps ----
        # Per sweep (512-q chunk, head pair): two A-halves (8 score pairs +
        # exps, pure 64-row PE tiling: head pairs run concurrently) and two
        # B-halves (16 AV matmuls + finalize transposes, pure 128-row
        # mode), interleaved across sweeps Ah1(n) Bh2(n-1) Ah2(n) Bh1(n)
        # so only 4 tiling-mode switches happen per sweep.
        class SweepK64:
            def __init__(self, qc, pair):
                self.qc, self.pair = qc, pair
                self.att = {}
                self.psy = None

            def A(self, half):
                qc, pair = self.qc, self.pair
                qT, kT = state["q", pair], state["k", pair]
                for kc in range(half * 8, half * 8 + 8):
                    pss = ps_pool.tile([128, NP, QCH], FP32, tag="big",
                                       name="pss")
                    for h in range(2):
                        nc.tensor.matmul(
                            pss[:, h, :],
                            lhsT=kT[h * HD:(h + 1) * HD,
                                    kc * 128:(kc + 1) * 128],
                            rhs=qT[h * HD:(h + 1) * HD,
                                   qc * QCH:(qc + 1) * QCH],
                            start=True, stop=True)
                    self.att[kc] = emit_exp(pss, [128, NP, QCH],
                                            kc in dve_kc)

            def B(self, half):
                qc, pair = self.qc, self.pair
                v65 = state["v65", pair]
                if self.psy is None:
                    self.psy = psy_pool.tile([65, NP, QCH], FP32,
                                             tag="psy", name="psy")
                for kc in range(half * 8, half * 8 + 8):
                    at_bf = self.att.pop(kc)
                    for h in range(2):
                        nc.tensor.matmul(
                            self.psy[:, h, :],
                            lhsT=v65[:, kc, h * 65:(h + 1) * 65],
                            rhs=at_bf[:, h, :],
                            start=(kc == 0), stop=(kc == NKC - 1))
                if half == 1:
                    self._finish()

            def _finish(self):
                qc, pair, psy = self.qc, self.pair, self.psy
                if ("yo", qc) not in state:
                    state["yo", qc] = yout_pool.tile(
                        [128, NJ, COLS], FP32, tag="yo", name="yo")
                yo = state["yo", qc]

                def make_ysb(h, ysb):
                    if h == 0:
                        nc.vector.tensor_copy(ysb[:], psy[:, h, :])
                    else:
                        nc.scalar.activation(
                            ysb[:], psy[:, h, :],
                            mybir.ActivationFunctionType.Copy)
                    return ysb

                finalize(pair, make_ysb, yo, NJ, QCH)
                if pair == NP - 1:
                    nc.sync.dma_start(
                        out=out[qc * QCH:(qc + 1) * QCH, :].rearrange(
                            "(j p) c -> p j c", p=128),
                        in_=yo[:])

        for _rep in range(reps):
            state.clear()
            emit_cast()
            emit_xt(0)
            emit_xt(1)
            if arch == "k64":
                # All 128-row-mode PE prologue work (projections, v
                # transposes), then half-phase pipelined sweeps.
                for pair in range(NP):
                    for p in ("k", "v", "q"):
                        for sc in range(NQC):
                            emit_proj(p, pair, sc)
                    emit_v65(pair, 0, NKC)
                prev = None
                for qc in range(NQC):
                    for pair in range(NP):
                        sw = SweepK64(qc, pair)
                        sw.A(0)
                        if prev is not None:
                            prev.B(1)
                        sw.A(1)
                        sw.B(0)
                        prev = sw
                prev.B(1)
                continue
            # Pair-0 prologue: k, v (for v65), q slice 0.
            for sc in range(NQC):
                emit_proj("k", 0, sc)
            for sc in range(NQC):
                emit_proj("v", 0, sc)
            emit_proj("q", 0, 0)
            emit_v65(0, 0, NKC)

            # Remaining prologue work, interleaved into attention sweeps.
            pending = []
            for sc in range(NQC):
                pending.append(("k1", lambda sc=sc: emit_proj("k", 1, sc)))
            for sc in range(NQC):
                pending.append(("v1", lambda sc=sc: emit_proj("v", 1, sc)))
            for kc0 in range(0, NKC, 4):
                pending.append(
                    ("v651", lambda kc0=kc0: emit_v65(1, kc0, 4)))
            pending.append(("q1s0", lambda: emit_proj("q", 1, 0)))
            pending.append(("q0s1", lambda: emit_proj("q", 0, 1)))
            pending.append(("q1s1", lambda: emit_proj("q", 1, 1)))
            pending.append(("q0s2", lambda: emit_proj("q", 0, 2)))
            pending.append(("q1s2", lambda: emit_proj("q", 1, 2)))
            pending.append(("q0s3", lambda: emit_proj("q", 0, 3)))
            pending.append(("q1s3", lambda: emit_proj("q", 1, 3)))
            pending.reverse()
            done = set()

            def hook():
                if pending:
                    key, fn = pending.pop()
                    fn()
                    done.add(key)

            def require_upto(key):
                # pending is in dependency order; popping up to `key`
                # drains every prerequisite before it too.
                while pending and key not in done:
                    hook()

            for qc in range(NQC):
                for pair in range(NP):
                    if pair == 1:
                        require_upto(f"q1s{qc}")
                    elif qc > 0:
                        require_upto(f"q0s{qc}")
                    emit_attn_qc(qc, pair, hook)
            while pending:
                hook()

    nc.compile()
    return nc


def get_nc(zero_bias=True):
    if zero_bias not in _CACHED_NC:
        _CACHED_NC[zero_bias] = build_nc(zero_bias=zero_bias)
    return _CACHED_NC[zero_bias]


def make_in_maps(x, wq, bq, wk, bk, wv, bv):
    in_maps = []
    for i in range(N_CORES):
        b = i // 4
        c0 = (i % 4) * COLS
        in_maps.append({
            "x": np.ascontiguousarray(x[b], dtype=np.float32),
            "wq": np.ascontiguousarray(wq[:, c0:c0 + COLS], dtype=np.float32),
            "wk": np.ascontiguousarray(wk[:, c0:c0 + COLS], dtype=np.float32),
            "wv": np.ascontiguousarray(wv[:, c0:c0 + COLS], dtype=np.float32),
            "bq": np.ascontiguousarray(bq[c0:c0 + COLS], dtype=np.float32),
            "bk": np.ascontiguousarray(bk[c0:c0 + COLS], dtype=np.float32),
            "bv": np.ascontiguousarray(bv[c0:c0 + COLS], dtype=np.float32),
        })
    return in_maps


def kernel(x, wq, bq, wk, bk, wv, bv):
    zero_bias = not (np.any(bq) or np.any(bk) or np.any(bv))
    nc = get_nc(zero_bias)
    in_maps = make_in_maps(x, wq, bq, wk, bk, wv, bv)
    res = run_bass_kernel_spmd(nc, in_maps, list(range(N_CORES)))
    out = np.empty((B, S, D), dtype=np.float32)
    for i in range(N_CORES):
        b = i // 4
        c0 = (i % 4) * COLS
        out[b, :, c0:c0 + COLS] = res.results[i]["out"]
    kernel.last_results = res
    return out
